# revision 36
# baseline (speedup 1.0000x reference)
"""GQA causal attention (B=2, T=2048, D=2048, N=16 q-heads, K=4 kv-heads, H=128)
on 8 Trainium2 NeuronCores.

Sharding: core c -> (batch b = c // 4, kv-head g = c % 4). Each core owns one
batch element and one GQA group (1 kv head + its 4 query heads) and computes
the full pipeline for that shard: Q/K/V projections, RoPE, causal SDPA, and
the O-projection partial over its 4 heads. The host pre-transposes activations
to [D, T] in bf16, precomputes RoPE sin/cos tables, and sums the 4 per-core
bf16 O-projection partials of each batch afterwards in fp64.

Device design notes:
  - All matmul operands in bf16 (PSUM accumulation stays fp32): 1 cycle/row
    at ANY free size (unlike fp32r which needs free>=256), half the DMA and
    SBUF traffic of fp32. End-to-end rel err ~4e-3 vs the 2e-2 gate.
  - qT/kT live [head_dim, T] (head dim on partitions); V is computed directly
    transposed [s, H] by swapping stationary/moving matmul operands (no PE
    transpose, no DVE copies); scores are computed transposed [s, t] so the
    softmax needs no on-chip transposes: exp without max-subtraction (logits
    are O(5) here), row sums via ones-vector matmuls accumulated in PSUM,
    reciprocal + Pool-engine partition_broadcast for the normalization.
  - Causal triangularity at 128-tile granularity: diagonal-chunk score/PV/
    row-sum matmuls only cover the t >= s_tile free-span (free sizes
    512/384/256/128); the causal mask is applied by zeroing the upper
    triangle of the diagonal probs block with a Pool-engine affine_select
    (no PSUM mask adds, nothing on the DVE critical path).
  - SDPA is software-pipelined over k-batches (scores/exp of batch b+1
    emitted before PV of batch b; first batch 3 deep) so PE never waits for
    the ACT engine's exp in steady state. K is projected before Q so RoPE-k
    on DVE hides under the Q matmuls; J0's Q projection is chunk-major while
    its xq stream lands, head-major after.
  - DMAs are consolidated with 3D access patterns and ordered so the first
    K-proj matmul starts ~1us in; output stores go through the ACT-triggered
    HWDGE queue to stay off the input-prefetch stream.
"""

import sys

for _p in ("/opt/trn_rl_repo", "/root/.axon_site/_ro/trn_rl_repo"):
    if _p not in sys.path:
        sys.path.append(_p)

import numpy as np

import concourse.bass as bass
import concourse.mybir as mybir
import concourse.tile as tile
from concourse import bacc
from concourse.bass_utils import run_bass_kernel_spmd

B, T, D = 2, 2048, 2048
N_HEADS, K_HEADS, H = 16, 4, 128
GH = N_HEADS // K_HEADS          # 4 query heads per core
MIN_TS, MAX_TS = 1.0, 10000.0
NJ = T // 512                    # 4 column chunks of 512
ND = D // 128                    # 16 contraction chunks
SCALE = 1.0 / float(np.sqrt(H))

F32 = mybir.dt.float32
MMDT = mybir.dt.bfloat16
NP_MM = mybir.dt.np(MMDT)

OG = 2    # 512-wide output blocks per store dma_start (1 or 2)

_CACHED_NC = None
_last_in_maps = None


def _build_core_program():
    nc = bacc.Bacc("TRN2", target_bir_lowering=False, debug=False, num_devices=8)

    xqT = nc.dram_tensor("xqT", [D, T], MMDT, kind="ExternalInput").ap()
    xkvT = nc.dram_tensor("xkvT", [D, T], MMDT, kind="ExternalInput").ap()
    wq = nc.dram_tensor("wq", [D, GH * H], MMDT, kind="ExternalInput").ap()
    wk = nc.dram_tensor("wk", [D, H], MMDT, kind="ExternalInput").ap()
    wv = nc.dram_tensor("wv", [D, H], MMDT, kind="ExternalInput").ap()
    wo = nc.dram_tensor("wo", [GH * H, D], MMDT, kind="ExternalInput").ap()
    tabs = nc.dram_tensor("tabs", [128, 4 * T], MMDT, kind="ExternalInput").ap()
    out = nc.dram_tensor("out", [T, D], MMDT, kind="ExternalOutput").ap()

    with tile.TileContext(nc) as tc:
        _emit(tc, nc, xqT, xkvT, wq, wk, wv, wo, tabs, out)
    nc.compile()
    return nc


def _emit(tc, nc, xqT, xkvT, wq, wk, wv, wo, tabs, out):
    from contextlib import ExitStack

    # 3D source views: [partition 128, d-chunk, col]
    xq_src = xqT.rearrange("(kd p) t -> p kd t", p=128)
    xkv_src = xkvT.rearrange("(kd p) t -> p kd t", p=128)
    wq_src = wq.rearrange("(kd p) n -> p kd n", p=128)
    wk_src = wk.rearrange("(kd p) n -> p kd n", p=128)
    wv_src = wv.rearrange("(kd p) n -> p kd n", p=128)
    wo_src = wo.rearrange("(h p) d -> p h d", p=128)
    tab_src = tabs.rearrange("p (i t) -> p i t", i=4)

    with ExitStack() as ctx:
        const = ctx.enter_context(tc.tile_pool(name="const", bufs=1))
        xq_pool = ctx.enter_context(tc.tile_pool(name="xq", bufs=1))
        xkv_pool = ctx.enter_context(tc.tile_pool(name="xkv", bufs=4))
        tab_pool = ctx.enter_context(tc.tile_pool(name="tab", bufs=2))
        qrot_pool = ctx.enter_context(tc.tile_pool(name="qrot", bufs=4))
        attnt_pool = ctx.enter_context(tc.tile_pool(name="attnt", bufs=4))
        probs_pool = ctx.enter_context(tc.tile_pool(name="probs", bufs=10))
        work = ctx.enter_context(tc.tile_pool(name="work", bufs=2))
        osb_pool = ctx.enter_context(tc.tile_pool(name="osb", bufs=2))
        psum = ctx.enter_context(tc.tile_pool(name="psum", bufs=5, space="PSUM"))
        psum_attn = ctx.enter_context(
            tc.tile_pool(name="psum_attn", bufs=2, space="PSUM")
        )
        psum_lrow = ctx.enter_context(
            tc.tile_pool(name="psum_lrow", bufs=1, space="PSUM")
        )

        ones_f = work.tile([128, 1], F32, tag="scratch", name="ones_f")
        nc.vector.memset(ones_f[:], 1.0)
        ones_s = const.tile([128, 1], MMDT, tag="ones_s")
        nc.vector.tensor_copy(ones_s[:], ones_f[:])
        krot_sb = const.tile([128, T], MMDT, tag="krot")
        v_sb = const.tile([128, T // 128, 128], MMDT, tag="v")  # [s-par, s-tile, h]
        wq_all = const.tile([128, ND, GH * H], MMDT, tag="wq")
        wk_all = const.tile([128, ND, H], MMDT, tag="wk")
        wv_all = const.tile([128, ND, H], MMDT, tag="wv")
        wo_all = const.tile([128, GH, D], MMDT, tag="wo")


        def rope(dst, src_psum, cc_t, ss_t):
            # dst = src * cc + swap_halves(src) * ss
            tmp1 = work.tile([128, 512], F32, tag="scratch")
            tmp2 = work.tile([128, 512], F32, tag="scratch")
            nc.vector.tensor_mul(tmp1[0:64, :], src_psum[64:128, :], ss_t[0:64, :])
            nc.vector.tensor_mul(tmp1[64:128, :], src_psum[0:64, :], ss_t[64:128, :])
            nc.vector.tensor_mul(tmp2[:], src_psum[:], cc_t[:])
            nc.vector.tensor_add(dst, tmp1[:], tmp2[:])

        for J in range(NJ):
            tsl = slice(J * 512, (J + 1) * 512)

            # DMA order is the J0 critical path: a tiny first chunk of wk and
            # xkv lets K-proj start ~1us in, then xkv / wq+xq quarters are
            # woven so both the K chain and the chunk-major Q chain stream.
            xkv_quarters = [
                xkv_pool.tile([128, 4, 512], MMDT, tag="xkv",
                              name=f"xkv{J}_{q}") for q in range(4)
            ]
            xq_t = xq_pool.tile([128, ND, 512], MMDT, tag="xq", name=f"xq{J}")
            if J == 0:
                nc.sync.dma_start(wk_all[:, 0:1, :], wk_src[:, 0:1, :])
                nc.sync.dma_start(xkv_quarters[0][:, 0:1, :],
                                  xkv_src[:, 0:1, tsl])
                nc.sync.dma_start(wk_all[:, 1:, :], wk_src[:, 1:, :])
                nc.sync.dma_start(xkv_quarters[0][:, 1:, :],
                                  xkv_src[:, 1:4, tsl])
                for q in range(4):
                    if q > 0:
                        nc.sync.dma_start(
                            xkv_quarters[q][:],
                            xkv_src[:, q * 4:(q + 1) * 4, tsl])
                    qs = slice(q * 4, (q + 1) * 4)
                    nc.sync.dma_start(wq_all[:, qs, :], wq_src[:, qs, :])
                    nc.sync.dma_start(xq_t[:, qs, :], xq_src[:, qs, tsl])
                    if q == 0:
                        tab_t = tab_pool.tile([128, 4, 512], MMDT, tag="tab",
                                              name=f"tab{J}")
                        nc.sync.dma_start(tab_t[:], tab_src[:, :, tsl])
                    if q == 1:
                        nc.sync.dma_start(wv_all[:], wv_src[:])
            else:
                for q in range(4):
                    nc.sync.dma_start(xkv_quarters[q][:],
                                      xkv_src[:, q * 4:(q + 1) * 4, tsl])
                tab_t = tab_pool.tile([128, 4, 512], MMDT, tag="tab",
                                      name=f"tab{J}")
                nc.sync.dma_start(tab_t[:], tab_src[:, :, tsl])
                for q in range(4):
                    qs = slice(q * 4, (q + 1) * 4)
                    nc.sync.dma_start(xq_t[:, qs, :], xq_src[:, qs, tsl])
            ccq_t, ssq_t = tab_t[:, 0, :], tab_t[:, 1, :]
            cck_t, ssk_t = tab_t[:, 2, :], tab_t[:, 3, :]

            # ---- K projection ([H, s]) first: its RoPE on DVE overlaps the
            # V/Q-projection matmuls below ----
            kps = psum.tile([128, 512], F32, tag="mm")
            for q in range(4):
                xkv_t = xkv_quarters[q]
                for kq in range(4):
                    kd = q * 4 + kq
                    nc.tensor.matmul(kps[:], wk_all[:, kd, :], xkv_t[:, kq, :],
                                     start=(kd == 0), stop=(kd == ND - 1),
                                     skip_group_check=True)
            rope(krot_sb[:, tsl], kps[:], cck_t, ssk_t)

            # ---- Q projection. At J0 the xq quarters land staggered, so run
            # chunk-major while DMA-gated (kd 0..11), then head-major so each
            # head's RoPE (DVE) starts as early as possible. J>0 is fully
            # head-major (xq is prefetched). ----
            qps_h = [psum.tile([128, 512], F32, tag="mm", name=f"qps{J}_{h}")
                     for h in range(GH)]
            qrot = []
            kd_cm = 12 if J == 0 else 0   # chunk-major prefix length
            for kd in range(kd_cm):
                for h in range(GH):
                    nc.tensor.matmul(
                        qps_h[h][:], wq_all[:, kd, h * 128:(h + 1) * 128],
                        xq_t[:, kd, :], start=(kd == 0), stop=False,
                        skip_group_check=True)
            for h in range(GH):
                for kd in range(kd_cm, ND):
                    nc.tensor.matmul(
                        qps_h[h][:], wq_all[:, kd, h * 128:(h + 1) * 128],
                        xq_t[:, kd, :], start=(kd == 0), stop=(kd == ND - 1),
                        skip_group_check=True)
                qr = qrot_pool.tile([128, 512], MMDT, tag="qrot",
                                    name=f"qrot{J}_{h}")
                rope(qr[:], qps_h[h][:], ccq_t, ssq_t)
                qrot.append(qr)

            # ---- V projection, directly transposed to [s, h]: stationary is
            # the xkv chunk [d, s-tile], moving is wv [d, h]. Emitted after Q
            # so these PE matmuls cover the last q-RoPE's DVE latency. ----
            vps = psum.tile([128, 512], F32, tag="mm")
            for st in range(4):
                for q in range(4):
                    xkv_t = xkv_quarters[q]
                    for kq in range(4):
                        kd = q * 4 + kq
                        nc.tensor.matmul(
                            vps[:, st * 128:(st + 1) * 128],
                            xkv_t[:, kq, st * 128:(st + 1) * 128],
                            wv_all[:, kd, :],
                            start=(kd == 0), stop=(kd == ND - 1),
                            skip_group_check=True)
            for st in range(4):
                nc.vector.tensor_copy(
                    v_sb[:, J * 4 + st, :], vps[:, st * 128:(st + 1) * 128])

            if J == 0:
                nc.sync.dma_start(wo_all[:], wo_src[:])

            # ---- SDPA for chunk J, all 4 heads ----
            # k-tile k covers keys [128k, 128k+128); for diagonal tiles
            # (k >= 4J) only the causal t-span [128*(k-4J), 512) is computed.
            attnT = []
            nk = 4 * J + 4
            for h in range(GH):
                attn_ps = psum_attn.tile([128, 512], F32, tag="attn",
                                         name=f"aps{J}_{h}")
                lrow_ps = psum_lrow.tile([1, 512], F32, tag="lrow")

                def emit_scores(k):
                    lo = max(0, (k - 4 * J) * 128)
                    span = 512 - lo
                    sc = psum.tile([128, 512], F32, tag="mm",
                                   name=f"sc{J}_{h}_{k}")
                    nc.tensor.matmul(sc[:, 0:span],
                                     krot_sb[:, k * 128:(k + 1) * 128],
                                     qrot[h][:, lo:512],
                                     start=True, stop=True)
                    pt = probs_pool.tile([128, 512], MMDT, tag="probs",
                                         name=f"pt{J}_{h}_{k}")
                    nc.scalar.activation(pt[:, 0:span], sc[:, 0:span],
                                         mybir.ActivationFunctionType.Exp,
                                         scale=SCALE)
                    if k >= 4 * J:
                        # causal mask: zero the upper triangle of the diag
                        # 128-block of pt on the idle Pool engine (keeps DVE
                        # and the PSUM scores off the mask's critical path)
                        nc.gpsimd.affine_select(
                            out=pt[:, 0:128], in_=pt[:, 0:128],
                            compare_op=mybir.AluOpType.is_ge,
                            fill=0.0, base=0,
                            pattern=[[1, 128]], channel_multiplier=-1)
                    return (k, lo, span, pt)

                def emit_pv(batch):
                    for k, lo, span, pt in batch:
                        nc.tensor.matmul(attn_ps[:, lo:512],
                                         v_sb[:, k, :], pt[:, 0:span],
                                         start=(k == 0), stop=(k == nk - 1),
                                         skip_group_check=True)
                        nc.tensor.matmul(lrow_ps[:, lo:512],
                                         ones_s[:], pt[:, 0:span],
                                         start=(k == 0), stop=(k == nk - 1),
                                         skip_group_check=True)

                # software pipeline over k-batches: scores/exp of batch b+1
                # are emitted before the PV/row-sum of batch b, so PE never
                # waits for ACT's exp in steady state. First batch is 3 deep
                # (then 2s) to cover the pipeline-fill exp latency; max live
                # sc tiles = 3 + 2 = 5 = the PSUM pool size.
                bounds = [0, min(3, nk)]
                while bounds[-1] < nk:
                    bounds.append(min(bounds[-1] + 2, nk))
                prev = None
                for b0, b1 in zip(bounds[:-1], bounds[1:]):
                    cur = [emit_scores(k) for k in range(b0, b1)]
                    if prev is not None:
                        emit_pv(prev)
                    prev = cur
                emit_pv(prev)

                # denominator: reciprocal on one row, Pool-engine broadcast
                # to 128 partitions (keeps it off PE and off a PSUM bank)
                lrow_sb = work.tile([1, 512], F32, tag="lrow", bufs=1,
                                    name=f"lrow_sb{J}_{h}")
                nc.vector.reciprocal_approx_fast(lrow_sb[:], lrow_ps[:])
                lbc_sb = work.tile([128, 512], F32, tag="scratch",
                                   name=f"lbc_sb{J}_{h}")
                nc.gpsimd.partition_broadcast(lbc_sb[:], lrow_sb[:])
                at = attnt_pool.tile([128, 512], MMDT, tag="attnt",
                                     name=f"at{J}_{h}")
                nc.vector.tensor_mul(at[:], attn_ps[:], lbc_sb[:])
                attnT.append(at)

            # ---- O projection for chunk J ----
            og = OG
            for tt in range(4):
                csl = slice(tt * 128, (tt + 1) * 128)
                for djp in range(4 // og):  # og d-chunks -> one DMA each
                    ot = osb_pool.tile([128, OG * 512], MMDT, tag="osb",
                                       name=f"ot{J}_{tt}_{djp}")
                    for dje in range(og):
                        dj = djp * og + dje
                        ops = psum.tile([128, 512], F32, tag="mm")
                        for h in range(GH):
                            nc.tensor.matmul(
                                ops[:], attnT[h][:, csl],
                                wo_all[:, h, dj * 512:(dj + 1) * 512],
                                start=(h == 0), stop=(h == GH - 1))
                        # alternate PSUM->SBUF copies between ACT and DVE so
                        # neither engine becomes the O-projection bottleneck
                        if dje % 2 == 0:
                            nc.vector.tensor_copy(
                                ot[:, dje * 512:(dje + 1) * 512], ops[:])
                        else:
                            nc.scalar.copy(
                                ot[:, dje * 512:(dje + 1) * 512], ops[:])
                    # ACT-triggered HWDGE: output stores stay out of the
                    # sync-engine input-prefetch stream
                    nc.scalar.dma_start(
                        out[J * 512 + tt * 128:J * 512 + (tt + 1) * 128,
                            djp * og * 512:(djp + 1) * og * 512],
                        ot[:, 0:og * 512])


def _rope_tables(positions):
    # positions: [T] int -> cc [128, T] = [cos; cos], ss [128, T] = [-sin; sin]
    half = H // 2
    fraction = 2.0 * np.arange(half, dtype=np.float64) / H
    timescale = MIN_TS * (MAX_TS / MIN_TS) ** fraction
    sinusoid = positions.astype(np.float64)[None, :] / timescale[:, None]
    sin = np.sin(sinusoid)
    cos = np.cos(sinusoid)
    cc = np.concatenate([cos, cos], axis=0).astype(np.float32)
    ss = np.concatenate([-sin, sin], axis=0).astype(np.float32)
    return cc, ss


def kernel(Xq, Xkv, q_positions, kv_positions, Wq, Wk, Wv, Wo):
    global _CACHED_NC, _last_in_maps
    if _CACHED_NC is None:
        _CACHED_NC = _build_core_program()
    nc = _CACHED_NC

    Xq = np.asarray(Xq, dtype=np.float32)
    Xkv = np.asarray(Xkv, dtype=np.float32)
    Wq = np.asarray(Wq, dtype=np.float32)
    Wk = np.asarray(Wk, dtype=np.float32)
    Wv = np.asarray(Wv, dtype=np.float32)
    Wo = np.asarray(Wo, dtype=np.float32)
    q_positions = np.asarray(q_positions)
    kv_positions = np.asarray(kv_positions)

    in_maps = []
    for c in range(8):
        b, g = c // 4, c % 4
        ccq, ssq = _rope_tables(q_positions[b])
        cck, ssk = _rope_tables(kv_positions[b])
        tabs = np.concatenate([ccq, ssq, cck, ssk], axis=1).astype(NP_MM)
        in_maps.append({
            "xqT": Xq[b].T.astype(NP_MM),
            "xkvT": Xkv[b].T.astype(NP_MM),
            "wq": Wq[:, g * GH:(g + 1) * GH, :].reshape(D, GH * H).astype(NP_MM),
            "wk": Wk[:, g, :].astype(NP_MM),
            "wv": Wv[:, g, :].astype(NP_MM),
            "wo": Wo[g * GH:(g + 1) * GH].reshape(GH * H, D).astype(NP_MM),
            "tabs": tabs,
        })

    _last_in_maps = in_maps

    res = run_bass_kernel_spmd(nc, in_maps, list(range(8)))

    outp = np.zeros((B, T, D), dtype=np.float64)
    for c in range(8):
        outp[c // 4] += res.results[c]["out"].astype(np.float64)
    return outp.astype(np.float32)
